# revision 39
# baseline (speedup 1.0000x reference)
"""Sliding-window causal GQA self-attention kernel for 8 Trainium2 NeuronCores.

Sharding: core c -> (batch b = c//4, kv-head g = c%4, q-heads 4g..4g+3).
Each core computes its 4 q-heads' attention and a partial output projection
(y_heads @ Wo[rows]); the host sums the 4 partials per batch.

v1 rewrite vs baseline:
- All matmul operands are bf16 (halves LDWEIGHTS time, halves DMA/SBUF
  traffic); accumulation stays fp32 in PSUM.
- All 4 q-heads are packed into the free dim of score/PV matmuls
  ([128, 4, 256] tiles, two 512-col matmuls per k-block) -> ~516 matmuls
  total vs 852, and the per-instruction LDWEIGHTS overhead shrinks.
- Scores no longer need the kdup duplicated-k trick: one 64-row k
  stationary serves all heads.
- Softmax denominator: ones-column in V (as before), then
  reciprocal_approx_fast (5x faster than DVE reciprocal) + gpsimd
  partition_broadcast instead of 4 slow single-partition RECIPROCALs +
  PE broadcast matmuls.
- Software-pipelined qb loop: scores(kb+1) are issued before PV(kb) so
  the PE never stalls on the scalar-engine exp.
- PSUM: sc/po share one 2-buf pool (4 banks) + yts double-buffered
  (4 banks) = exactly 8 banks.
"""

import numpy as np

import concourse.bass as bass
import concourse.mybir as mybir
import concourse.tile as tile
from concourse.bass import ds, ts

F32 = mybir.dt.float32
BF = mybir.dt.bfloat16
AF = mybir.ActivationFunctionType

B, T, NE = 2, 2048, 1024
NH, NKV, HD = 16, 4, 64
GC = 32
WIN = 1024
EPS = 1e-6
BIG = 1.0e9
NCORES = 8
QB = 256          # q-block (free dim per head of QK/PV matmuls)
NQB = T // QB     # 8
NKB = T // 128    # 16 k-blocks
SCALE = 1.0 / 8.0  # 1/sqrt(HD)

# Skip the LDWEIGHTS for a matmul whose stationary operand is already loaded
# (the previous PE instruction used the same lhsT). Saves ~150-200ns of PE
# drain+reload per elided matmul.
ELIDE_LDW = True


def _no_ldw(mi):
    if ELIDE_LDW:
        mi.ins.ldweights = False
    return mi


def _build_nc():
    nc = bass.Bass(trn_type="TRN2", target_bir_lowering=False)

    d = {}
    for name, shape, dt in [
        ("xT", (NE, T), BF), ("ve", (T, HD), BF),
        ("cos4", (128, T), BF), ("sin4", (128, T), BF),
        ("coskv", (128, T), BF), ("sinkv", (128, T), BF),
        ("wq", (NE, 256), BF), ("wkv", (NE, 128), BF), ("wg", (GC, 1), BF),
        ("wo", (256, NE), BF),
        ("pswq", (128, 128), BF), ("pswkv", (128, 128), BF),
        ("bdq", (128, 2), BF), ("bdk", (128, 1), BF),
        ("e2sel", (2, 128), BF), ("ident", (128, 128), BF),
        ("triA", (128, 128), BF), ("triA2", (128, 128), BF),
        ("bc0", (128, 2 * QB), BF), ("bc1", (128, 2 * QB), BF),
        ("bw0", (128, 2 * QB), BF), ("bw1", (128, 2 * QB), BF),
        ("ones64", (1, 64), BF), ("ones64c", (64, 1), BF),
    ]:
        d[name] = nc.dram_tensor(name, list(shape), dt, kind="ExternalInput")
    out_d = nc.dram_tensor("out", [T, NE], F32, kind="ExternalOutput")

    with tile.TileContext(nc) as tc:
        with (
            nc.allow_low_precision(reason="bf16 compute, fp32 accumulate"),
            tc.tile_pool(name="persist", bufs=1) as pp,
            tc.tile_pool(name="smalls", bufs=4) as sm,
        ):
            # ---- persistent tiles ----
            qall = pp.tile([64, 4, T], BF, tag="qall", name="qall")
            kvfin = pp.tile([128, T], BF, tag="kvfin", name="kvfin")
            rskt = pp.tile([128, NKB], F32, tag="rskt", name="rskt")
            vaug = [pp.tile([128, HD + 1], BF, tag=f"vaug{k}", name=f"vaug{k}")
                    for k in range(NKB)]
            ytall = [pp.tile([128, T], BF, tag=f"ytall{i}", name=f"ytall{i}")
                     for i in range(2)]
            wo_sb = [pp.tile([128, NE], BF, tag=f"wo{i}", name=f"wo{i}")
                     for i in range(2)]
            cst = {}
            for nm, shp in [("e2sel", [2, 128]), ("ident", [128, 128]),
                            ("triA", [128, 128]), ("triA2", [128, 128]),
                            ("bc0", [128, 2 * QB]), ("bc1", [128, 2 * QB]),
                            ("bw0", [128, 2 * QB]), ("bw1", [128, 2 * QB]),
                            ("ones64", [1, 64]), ("ones64c", [64, 1])]:
                cst[nm] = pp.tile(shp, BF, tag=nm, name=nm)
            # Phase-A constants now; attention-only constants (masks, wo)
            # are DMA'd at the start of Phase B to keep the head free for x
            for nm in ("e2sel", "ident", "ones64c"):
                nc.sync.dma_start(cst[nm][:], d[nm][:])
            eps_sb = pp.tile([128, 1], F32, tag="eps")
            nc.vector.memset(eps_sb[:], EPS)
            lnsc_sb = pp.tile([128, 1], F32, tag="lnsc")
            nc.vector.memset(lnsc_sb[:], float(np.log(SCALE)))

            # =================================================================
            # Phase A: projections + rope + rmsnorm + vaug build
            # =================================================================
            with (
                tc.tile_pool(name="xp", bufs=1) as xp,
                tc.tile_pool(name="work", bufs=1) as wk,
                tc.tile_pool(name="trig", bufs=1) as trg,
                tc.tile_pool(name="pj_ps", bufs=2, space="PSUM") as pjp,
                tc.tile_pool(name="sw_ps", bufs=2, space="PSUM") as swp,
                tc.tile_pool(name="aux_ps", bufs=1, space="PSUM") as axp,
            ):
                xsb = [xp.tile([128, T], BF, tag=f"x{e}", name=f"x{e}")
                       for e in range(8)]
                wq_sb = [xp.tile([128, 256], BF, tag=f"wq{e}", name=f"wqs{e}")
                         for e in range(8)]
                wkv_sb = [xp.tile([128, 128], BF, tag=f"wkv{e}", name=f"wkvs{e}")
                          for e in range(8)]
                # weights + x first, split into [32, T] row-slices so all 16
                # DMA queues load-balance and x lands as early as possible
                for e in range(8):
                    nc.sync.dma_start(wq_sb[e][:], d["wq"][ds(128 * e, 128), :])
                    nc.sync.dma_start(wkv_sb[e][:], d["wkv"][ds(128 * e, 128), :])
                wg_sb = sm.tile([GC, 1], BF, tag="wg")
                nc.sync.dma_start(wg_sb[:], d["wg"][:])
                aux = {}
                for nm, shp in [("pswq", [128, 128]), ("pswkv", [128, 128]),
                                ("bdq", [128, 2]), ("bdk", [128, 1])]:
                    aux[nm] = xp.tile(shp, BF, tag=nm, name=f"aux_{nm}")
                    nc.sync.dma_start(aux[nm][:], d[nm][:])
                for e in range(8):
                    for h in range(4):
                        rows = ds(32 * h, 32)
                        nc.sync.dma_start(xsb[e][rows, :],
                                          d["xT"][ds(128 * e + 32 * h, 32), :])
                ve_sb = xp.tile([128, NKB, HD], BF, tag="ve")
                nc.sync.dma_start(
                    ve_sb[:], d["ve"][:, :].rearrange("(n p) d -> p n d", p=128))

                # gate: u = x[:, :GC] @ wg ; g2 = sigmoid(u) (ve pre-doubled)
                gate_ps = axp.tile([128, NKB], F32, tag="aux")
                for kb in range(NKB):
                    nc.tensor.matmul(
                        gate_ps[:, ds(kb, 1)],
                        xsb[0][0:GC, ts(kb, 128)], wg_sb[:],
                        start=True, stop=True)
                g2 = xp.tile([128, NKB], F32, tag="g2")
                nc.scalar.activation(g2[:], gate_ps[:], AF.Sigmoid)

                # Phase A is software-pipelined across the three projection
                # calls (kv, q-pair0, q-pair1): stage_a is the big PE block
                # (projection + rope swap matmuls); the DVE/scalar-heavy
                # rms + scale tails hide under the next call's stage_a.
                # The k rmsnorm never touches k itself: it is folded into the
                # exp() of Phase B as a per-k-token (per-partition) scale.
                def stage_a(widx, w_tiles, mcols, psw, cos_t, sin_t,
                            raw=None, sq_rows=128):
                    if raw is None:
                        raw = wk.tile([128, T], BF, tag=f"w0{widx}", bufs=1,
                                      name=f"raw{widx}")
                    t1 = wk.tile([128, T], BF, tag=f"w1{widx}", bufs=1,
                                 name=f"t1{widx}")
                    tmp2 = wk.tile([128, T], BF, tag=f"w2{widx}", bufs=1,
                                   name=f"tmp2{widx}")
                    for nchk in range(4):
                        cols = ds(512 * nchk, 512)
                        ps = pjp.tile([128, 512], F32, tag="pj")
                        for e in range(8):
                            nc.tensor.matmul(
                                ps[:], w_tiles[e][:, mcols],
                                xsb[e][:, cols],
                                start=(e == 0), stop=(e == 7))
                        nc.any.tensor_copy(raw[:, cols], ps[:])
                    # rope: roped = raw*cos + (psw @ raw)*sin   (in place)
                    nc.vector.tensor_mul(t1[:], raw[:], cos_t[:])
                    for nchk in range(4):
                        cols = ds(512 * nchk, 512)
                        sw = swp.tile([128, 512], F32, tag="sw")
                        mi = nc.tensor.matmul(sw[:], psw[:], raw[:, cols],
                                              start=True, stop=True)
                        if nchk > 0:
                            _no_ldw(mi)
                        nc.vector.tensor_mul(tmp2[:, cols], sw[:],
                                             sin_t[:, cols])
                    roped = raw
                    nc.vector.tensor_add(roped[:], t1[:], tmp2[:])
                    sq = t1
                    nc.vector.tensor_mul(sq[0:sq_rows, :],
                                         roped[0:sq_rows, :],
                                         roped[0:sq_rows, :])
                    return roped, sq

                def k_scale(kv_sq):
                    """rskt[tok%128, tok//128] = SCALE*rsqrt(mean k^2 + eps),
                    token-major so it can feed exp()'s per-partition scale."""
                    msk = axp.tile([128, NKB], F32, tag="aux")
                    for kb in range(NKB):
                        nc.tensor.matmul(
                            msk[:, ds(kb, 1)],
                            kv_sq[0:64, ts(kb, 128)], cst["ones64c"][:],
                            start=True, stop=True)
                    lnk = sm.tile([128, NKB], F32, tag="lnk", bufs=1)
                    nc.scalar.activation(lnk[:], msk[:], AF.Ln,
                                         scale=1.0 / HD, bias=eps_sb[:])
                    nc.scalar.activation(rskt[:], lnk[:], AF.Exp,
                                         scale=-0.5, bias=lnsc_sb[:])
                    return rskt

                def stage_bc_q(i, roped, sq):
                    """per-512-chunk: rms stats -> rsqrt row -> broadcast ->
                    scaled bf16 heads into qall (chunk-pipelined)."""
                    for nchk in range(4):
                        cols = ds(512 * nchk, 512)
                        msps = axp.tile([2, 512], F32, tag="aux")
                        nc.tensor.matmul(msps[:], aux["bdq"][:, 0:2],
                                         sq[:, cols], start=True, stop=True)
                        lnm = sm.tile([2, 512], F32, tag="lnm", bufs=2)
                        nc.scalar.activation(lnm[:], msps[:], AF.Ln,
                                             scale=1.0 / HD,
                                             bias=eps_sb[0:2, :])
                        rsc = sm.tile([2, 512], BF, tag="rsc", bufs=2)
                        nc.scalar.activation(rsc[:], lnm[:], AF.Exp,
                                             scale=-0.5)
                        rsb = swp.tile([128, 512], F32, tag="sw")
                        nc.tensor.matmul(rsb[:], cst["e2sel"][:], rsc[:],
                                         start=True, stop=True)
                        for hl in range(2):
                            nc.vector.tensor_mul(
                                qall[:, 2 * i + hl, cols],
                                roped[ds(64 * hl, 64), cols],
                                rsb[ds(64 * hl, 64), :])

                def build_vaug():
                    for kb in range(NKB):
                        vt = pjp.tile([128, HD], BF, tag="pj", bufs=2)
                        nc.tensor.transpose(vt[:], kvfin[64:128, ts(kb, 128)],
                                            cst["ident"][64:128, 64:128])
                        gv = sm.tile([128, HD], BF, tag="gv")
                        nc.vector.tensor_scalar_mul(gv[:], ve_sb[:, kb, :],
                                                    g2[:, ds(kb, 1)])
                        nc.vector.memset(vaug[kb][:, ds(HD, 1)], 1.0)
                        nc.vector.tensor_add(vaug[kb][:, 0:HD], gv[:], vt[:])

                cos_kv = trg.tile([128, T], BF, tag="tckv")
                sin_kv = trg.tile([128, T], BF, tag="tskv")
                cos_q = trg.tile([128, T], BF, tag="tcq")
                sin_q = trg.tile([128, T], BF, tag="tsq")
                for tile_, nm in [(cos_kv, "coskv"), (sin_kv, "sinkv"),
                                  (cos_q, "cos4"), (sin_q, "sin4")]:
                    for h in range(4):
                        rows = ds(32 * h, 32)
                        nc.sync.dma_start(tile_[rows, :], d[nm][rows, :])

                kv_roped, kv_sq = stage_a(2, wkv_sb, ds(0, 128),
                                          aux["pswkv"], cos_kv, sin_kv,
                                          raw=kvfin, sq_rows=64)
                q0_roped, q0_sq = stage_a(0, wq_sb, ds(0, 128),
                                          aux["pswq"], cos_q, sin_q)
                k_scale(kv_sq)
                build_vaug()
                q1_roped, q1_sq = stage_a(1, wq_sb, ds(128, 128),
                                          aux["pswq"], cos_q, sin_q)
                stage_bc_q(0, q0_roped, q0_sq)
                stage_bc_q(1, q1_roped, q1_sq)

            # =================================================================
            # Phase B: attention + output projection
            # =================================================================
            with (
                tc.tile_pool(name="big_ps", bufs=2, space="PSUM") as bigp,
                tc.tile_pool(name="yt_ps", bufs=2, space="PSUM") as ytp,
                tc.tile_pool(name="et", bufs=4) as etp,
                tc.tile_pool(name="stage", bufs=2) as stg,
            ):
                for nm in ("triA", "triA2", "bc0", "bc1", "bw0", "bw1",
                           "ones64"):
                    nc.sync.dma_start(cst[nm][:], d[nm][:])
                for i in range(2):
                    nc.sync.dma_start(wo_sb[i][:], d["wo"][ds(128 * i, 128), :])
                def mask_for(qb, kb):
                    if kb == 2 * qb:
                        return (cst["triA"], cst["bc0"])
                    if kb == 2 * qb + 1:
                        return (cst["triA"], cst["bc1"])
                    if kb == 2 * qb - 8:
                        return (cst["triA2"], cst["bw0"])
                    if kb == 2 * qb - 7:
                        return (cst["triA2"], cst["bw1"])
                    return None

                def make_tail(qb, yts):
                    """Denominator + ytall writes for qb (emitted inside the
                    next qb's score stream so the PE never idles on it)."""
                    qsl = ds(QB * qb, QB)

                    def tail():
                        # Z row -> bf16 SBUF, PE-broadcast to 64 partitions,
                        # 1/Z = exp(-ln(Z)) on the lane-parallel scalar engine
                        zrow = sm.tile([1, 4, QB], BF, tag="zrow", bufs=2)
                        nc.vector.tensor_copy(zrow[:], yts[ds(HD, 1), :, :])
                        rbt = bigp.tile([128, 4, QB], F32, tag="big",
                                        name=f"rb{qb}")
                        for p in range(2):
                            mi = nc.tensor.matmul(rbt[0:64, ds(2 * p, 2), :],
                                                  cst["ones64"][:],
                                                  zrow[:, ds(2 * p, 2), :],
                                                  start=True, stop=True)
                            if p == 1:
                                _no_ldw(mi)
                        lnz = stg.tile([64, 4, QB], F32, tag="lnz", bufs=2)
                        nc.scalar.activation(lnz[:], rbt[0:64, :, :], AF.Ln)
                        rinv = stg.tile([64, 4, QB], F32, tag="rinv", bufs=2)
                        nc.scalar.activation(rinv[:], lnz[:], AF.Exp,
                                             scale=-1.0)
                        for h in range(4):
                            nc.vector.tensor_mul(
                                ytall[h // 2][ds(64 * (h % 2), 64), qsl],
                                yts[0:HD, h, :], rinv[:, h, :])

                    def outp(tt):
                        po = bigp.tile([128, 4, QB], F32, tag="big",
                                       name=f"po{tt}")
                        for i in range(2):
                            for nn in range(2):
                                mi = nc.tensor.matmul(
                                    po[:, ds(2 * nn, 2), :],
                                    ytall[i][:, ts(tt, 128)],
                                    wo_sb[i][:, ds(512 * nn, 512)],
                                    start=(i == 0), stop=(i == 1),
                                    skip_group_check=True)
                                if nn == 1:
                                    _no_ldw(mi)
                        osb = stg.tile([128, 4, QB], F32, tag="osb", bufs=2)
                        nc.vector.tensor_copy(osb[:], po[:])
                        nc.sync.dma_start(
                            out_d[ts(tt, 128), :].rearrange(
                                "p (n c) -> p n c", n=4), osb[:])

                    return [tail, lambda: outp(2 * qb),
                            lambda: outp(2 * qb + 1)]

                pending = []
                for qb in range(NQB):
                    kbs = list(range(max(0, 2 * qb - 8), 2 * qb + 2))
                    qsl = ds(QB * qb, QB)
                    yts = ytp.tile([HD + 1, 4, QB], F32, tag="yts",
                                   name=f"yts{qb}")

                    def emit_scores(kb):
                        sc = bigp.tile([128, 4, QB], F32, tag="big")
                        mask = mask_for(qb, kb)
                        for p in range(2):
                            mi = nc.tensor.matmul(
                                sc[:, ds(2 * p, 2), :],
                                kvfin[0:64, ts(kb, 128)],
                                qall[:, ds(2 * p, 2), qsl],
                                start=True, stop=(mask is None),
                                skip_group_check=True)
                            if p == 1:
                                _no_ldw(mi)
                        if mask is not None:
                            for p in range(2):
                                mi = nc.tensor.matmul(
                                    sc[:, ds(2 * p, 2), :],
                                    mask[0][:], mask[1][:],
                                    start=False, stop=True,
                                    skip_group_check=True)
                                if p == 1:
                                    _no_ldw(mi)
                        et = etp.tile([128, 4, QB], BF, tag="et")
                        nc.scalar.activation(et[:], sc[:], AF.Exp,
                                             scale=rskt[:, ds(kb, 1)])
                        return et

                    def emit_pv(kb, et):
                        for p in range(2):
                            mi = nc.tensor.matmul(
                                yts[:, ds(2 * p, 2), :], vaug[kb][:],
                                et[:, ds(2 * p, 2), :],
                                start=(kb == kbs[0]), stop=(kb == kbs[-1]))
                            if p == 1:
                                _no_ldw(mi)

                    # depth-1 PV pipeline within qb; the previous qb's tail
                    # and output projections flush between early units here
                    window = []
                    for idx, kb in enumerate(kbs):
                        et = emit_scores(kb)
                        if 1 <= idx <= 3 and pending:
                            pending.pop(0)()
                        window.append((kb, et))
                        if len(window) > 1:
                            emit_pv(*window.pop(0))
                    while pending:
                        pending.pop(0)()
                    for unit in window:
                        emit_pv(*unit)
                    pending = make_tail(qb, yts)
                for fn in pending:
                    fn()

    return nc


# ---------------------------------------------------------------------------
# walrus workaround: this build rejects >1 sync-wait on CTRL-class ops
# (e.g. the Tile tail Drain). Move excess waits onto NOPs inserted before.
# ---------------------------------------------------------------------------
_CTRL_TYPES = (mybir.InstDrain, mybir.InstNoOp, mybir.InstEventSemaphore)


def _split_excess_waits(nc, limit=1):
    for fn in nc.m.functions:
        for bb in fn.blocks:
            out, changed = [], False
            for inst in bb.instructions:
                si = inst.sync_info
                waits = list(si.on_wait) if si is not None and si.on_wait else []
                if len(waits) > limit:
                    extra, keep = waits[:-limit], waits[-limit:]
                    while extra:
                        chunk, extra = extra[:limit], extra[limit:]
                        nop = mybir.InstNoOp(
                            name=f"{inst.name}-wsplit{len(out)}", ins=[],
                            outs=[])
                        nop.engine = inst.engine
                        nop.sync_info = mybir.SyncInfo(on_wait=chunk,
                                                       on_update=[])
                        out.append(nop)
                    si.on_wait = keep
                    inst.sync_info = si
                    changed = True
                out.append(inst)
            if changed:
                bb.instructions = out


# ---------------------------------------------------------------------------
# Host-side constants (shared by all cores)
# ---------------------------------------------------------------------------
_BF_NP = mybir.dt.np(BF)


def _bf(a):
    return np.asarray(a, dtype=_BF_NP)


def _host_constants():
    c = {}
    m = np.arange(128)[:, None]
    j = np.arange(128)[None, :]
    i = np.arange(QB)[None, :]
    c["triA"] = _bf(m <= j)                          # causal counting lhsT
    c["triA2"] = _bf(m >= j)                         # window counting lhsT
    bc0 = np.where(m > i, -BIG, 0.0)
    bc1 = np.where(m > i - 128, -BIG, 0.0)
    bw0 = np.where(m < i, -BIG, 0.0)
    bw1 = np.where(m + 128 < i, -BIG, 0.0)
    for nm, v in [("bc0", bc0), ("bc1", bc1), ("bw0", bw0), ("bw1", bw1)]:
        c[nm] = _bf(np.tile(v, (1, 2)))              # 2 heads per matmul
    sw = np.zeros((128, 128), np.float32)            # pswq[f, m]=1 iff f=sig(m)
    for mm in range(128):
        f = mm + 32 if (mm % 64) < 32 else mm - 32
        sw[f, mm] = 1.0
    c["pswq"] = _bf(sw)
    swkv = sw.copy()
    swkv[:, 64:] = 0.0
    c["pswkv"] = _bf(swkv)
    bdq = np.zeros((128, 2), np.float32)
    bdq[0:64, 0] = 1.0
    bdq[64:128, 1] = 1.0
    c["bdq"] = _bf(bdq)
    bdk = np.zeros((128, 1), np.float32)
    bdk[0:64, 0] = 1.0
    c["bdk"] = _bf(bdk)
    e2 = np.zeros((2, 128), np.float32)
    e2[0, 0:64] = 1.0
    e2[1, 64:128] = 1.0
    c["e2sel"] = _bf(e2)
    c["ident"] = _bf(np.eye(128))
    c["ones64"] = _bf(np.ones((1, 64)))
    c["ones64c"] = _bf(np.ones((64, 1)))
    return c


def _trig(cos_b, sin_b):
    """cos_b/sin_b: [T, HD//2] -> the four [128, T] rope coefficient maps."""
    ct = np.ascontiguousarray(cos_b.T)               # [32, T]
    st = np.ascontiguousarray(sin_b.T)
    cos4 = np.tile(ct, (4, 1))                       # [c;c;c;c]
    sin4 = np.tile(np.concatenate([st, -st], 0), (2, 1))
    coskv = np.concatenate([ct, ct, np.ones((64, T), np.float32)], 0)
    sinkv = np.concatenate([st, -st, np.zeros((64, T), np.float32)], 0)
    return _bf(cos4), _bf(sin4), _bf(coskv), _bf(sinkv)


# ---------------------------------------------------------------------------
# Cached PJRT runner (compile once per process)
# ---------------------------------------------------------------------------
_RUNNER = None


def _get_runner():
    global _RUNNER
    if _RUNNER is not None:
        return _RUNNER
    import jax
    from jax.experimental.shard_map import shard_map
    from jax.sharding import Mesh, PartitionSpec
    from concourse.bass2jax import (_bass_exec_p, install_neuronx_cc_hook,
                                    partition_id_tensor)

    nc = _build_nc()
    _split_excess_waits(nc)
    install_neuronx_cc_hook()

    pid_name = (nc.partition_id_tensor.name
                if nc.partition_id_tensor is not None else None)
    in_names, out_names, out_avals, zero_outs = [], [], [], []
    for alloc in nc.m.functions[0].allocations:
        if not isinstance(alloc, mybir.MemoryLocationSet):
            continue
        name = alloc.memorylocations[0].name
        if alloc.kind == "ExternalInput":
            if name == pid_name:
                continue
            in_names.append(name)
        elif alloc.kind == "ExternalOutput":
            np_dt = mybir.dt.np(alloc.dtype)
            out_names.append(name)
            out_avals.append(
                jax.core.ShapedArray(tuple(alloc.tensor_shape), np_dt))
            zero_outs.append(
                np.zeros(tuple(alloc.tensor_shape), np_dt))

    def _body(*args):
        operands = list(args)
        if pid_name is not None:
            operands.append(partition_id_tensor())
        outs = _bass_exec_p.bind(
            *operands,
            out_avals=tuple(out_avals),
            in_names=(tuple(in_names) + tuple(out_names)
                      + ((pid_name,) if pid_name else ())),
            out_names=tuple(out_names),
            lowering_input_output_aliases=(),
            sim_require_finite=True,
            sim_require_nnan=True,
            nc=nc,
        )
        return tuple(outs)

    devices = jax.devices()[:NCORES]
    mesh = Mesh(np.asarray(devices), ("core",))
    n_args = len(in_names) + len(out_names)
    sharded = jax.jit(
        shard_map(_body, mesh=mesh,
                  in_specs=(PartitionSpec("core"),) * n_args,
                  out_specs=(PartitionSpec("core"),) * len(out_names),
                  check_rep=False),
        keep_unused=True,
    )

    def run(in_maps):
        concat_in = [
            np.concatenate([in_maps[c][nm] for c in range(NCORES)], axis=0)
            for nm in in_names
        ]
        concat_zero = [
            np.zeros((NCORES * z.shape[0], *z.shape[1:]), z.dtype)
            for z in zero_outs
        ]
        outs = sharded(*concat_in, *concat_zero)
        res = []
        for c in range(NCORES):
            res.append({
                nm: np.asarray(outs[i]).reshape(NCORES, *out_avals[i].shape)[c]
                for i, nm in enumerate(out_names)
            })
        return res

    _RUNNER = {"run": run, "sharded": sharded, "in_names": in_names,
               "out_names": out_names, "out_avals": out_avals,
               "zero_outs": zero_outs, "nc": nc, "mesh": mesh}
    return _RUNNER


def _make_in_maps(x, ve, cos, sin, Wq, Wk, Wv, Wo, Wg):
    cstc = _host_constants()
    in_maps = []
    for c in range(NCORES):
        b, g = c // 4, c % 4
        cos4, sin4, coskv, sinkv = _trig(np.asarray(cos[b]),
                                         np.asarray(sin[b]))
        m = {
            "xT": _bf(np.asarray(x[b]).T),
            # gate = 2*sigmoid(..): the 2x is folded into ve here
            "ve": _bf(2.0 * np.asarray(ve[b])[:, HD * g:HD * (g + 1)]),
            "cos4": cos4, "sin4": sin4, "coskv": coskv, "sinkv": sinkv,
            "wq": _bf(Wq[:, 256 * g:256 * (g + 1)]),
            "wkv": _bf(np.concatenate([Wk[:, HD * g:HD * (g + 1)],
                                       Wv[:, HD * g:HD * (g + 1)]], axis=1)),
            "wg": _bf(Wg[:, g:g + 1]),
            "wo": _bf(Wo[256 * g:256 * (g + 1), :]),
        }
        m.update(cstc)
        in_maps.append(m)
    return in_maps


def kernel(x, ve, cos, sin, Wq, Wk, Wv, Wo, Wg, window_size):
    assert int(window_size) == WIN, f"kernel hardcodes window={WIN}"
    x, ve, cos, sin = (np.asarray(a, np.float32) for a in (x, ve, cos, sin))
    Wq, Wk, Wv, Wo, Wg = (np.asarray(a, np.float32)
                          for a in (Wq, Wk, Wv, Wo, Wg))
    runner = _get_runner()
    in_maps = _make_in_maps(x, ve, cos, sin, Wq, Wk, Wv, Wo, Wg)
    res = runner["run"](in_maps)
    out = np.zeros((B, T, NE), np.float32)
    for c in range(NCORES):
        out[c // 4] += res[c]["out"]
    return out


# revision 47
# speedup vs baseline: 1.1302x; 1.1302x over previous
"""Sliding-window causal GQA self-attention kernel for 8 Trainium2 NeuronCores.

Sharding: core c -> (batch b = c//4, kv-head g = c%4, q-heads 4g..4g+3).
Each core computes its 4 q-heads' attention and a partial output projection
(y_heads @ Wo[rows]); the host sums the 4 partials per batch.

v1 rewrite vs baseline:
- All matmul operands are bf16 (halves LDWEIGHTS time, halves DMA/SBUF
  traffic); accumulation stays fp32 in PSUM.
- All 4 q-heads are packed into the free dim of score/PV matmuls
  ([128, 4, 256] tiles, two 512-col matmuls per k-block) -> ~516 matmuls
  total vs 852, and the per-instruction LDWEIGHTS overhead shrinks.
- Scores no longer need the kdup duplicated-k trick: one 64-row k
  stationary serves all heads.
- Softmax denominator: ones-column in V (as before), then
  reciprocal_approx_fast (5x faster than DVE reciprocal) + gpsimd
  partition_broadcast instead of 4 slow single-partition RECIPROCALs +
  PE broadcast matmuls.
- Software-pipelined qb loop: scores(kb+1) are issued before PV(kb) so
  the PE never stalls on the scalar-engine exp.
- PSUM: sc/po share one 2-buf pool (4 banks) + yts double-buffered
  (4 banks) = exactly 8 banks.
"""

import numpy as np

import concourse.bass as bass
import concourse.mybir as mybir
import concourse.tile as tile
from concourse.bass import ds, ts

F32 = mybir.dt.float32
BF = mybir.dt.bfloat16
AF = mybir.ActivationFunctionType

B, T, NE = 2, 2048, 1024
NH, NKV, HD = 16, 4, 64
GC = 32
WIN = 1024
EPS = 1e-6
BIG = 1.0e9
NCORES = 8
QB = 256          # q-block (free dim per head of QK/PV matmuls)
NQB = T // QB     # 8
NKB = T // 128    # 16 k-blocks
SCALE = 1.0 / 8.0  # 1/sqrt(HD)

# Skip the LDWEIGHTS for a matmul whose stationary operand is already loaded
# (the previous PE instruction used the same lhsT). Saves ~150-200ns of PE
# drain+reload per elided matmul.
ELIDE_LDW = False


def _no_ldw(mi):
    if ELIDE_LDW:
        mi.ins.ldweights = False
    return mi


def _build_nc():
    nc = bass.Bass(trn_type="TRN2", target_bir_lowering=False)

    d = {}
    for name, shape, dt in [
        ("xT", (NE, T), BF), ("ve", (T, HD), BF),
        ("cos4", (128, T), BF), ("sin4", (128, T), BF),
        ("coskv", (128, T), BF), ("sinkv", (128, T), BF),
        ("wq", (NE, 256), BF), ("wkv", (NE, 128), BF), ("wg", (GC, 1), BF),
        ("wo", (256, NE), BF),
        ("pswq", (128, 128), BF), ("pswkv", (128, 128), BF),
        ("bdq", (128, 2), BF), ("bdk", (128, 1), BF),
        ("e2sel", (2, 128), BF), ("ident", (128, 128), BF),
        ("triA", (128, 128), BF), ("triA2", (128, 128), BF),
        ("bc0", (128, 2 * QB), BF), ("bc1", (128, 2 * QB), BF),
        ("bw0", (128, 2 * QB), BF), ("bw1", (128, 2 * QB), BF),
        ("ones64", (1, 64), BF), ("ones64c", (64, 1), BF),
    ]:
        d[name] = nc.dram_tensor(name, list(shape), dt, kind="ExternalInput")
    out_d = nc.dram_tensor("out", [T, NE], F32, kind="ExternalOutput")

    with tile.TileContext(nc) as tc:
        with (
            nc.allow_low_precision(reason="bf16 compute, fp32 accumulate"),
            tc.tile_pool(name="persist", bufs=1) as pp,
            tc.tile_pool(name="smalls", bufs=4) as sm,
        ):
            # ---- persistent tiles ----
            qall = pp.tile([64, 4, T], BF, tag="qall", name="qall")
            kvfin = pp.tile([128, T], BF, tag="kvfin", name="kvfin")
            vaug = [pp.tile([128, HD + 1], BF, tag=f"vaug{k}", name=f"vaug{k}")
                    for k in range(NKB)]
            ytall = [pp.tile([128, T], BF, tag=f"ytall{i}", name=f"ytall{i}")
                     for i in range(2)]
            wo_sb = [pp.tile([128, NE], BF, tag=f"wo{i}", name=f"wo{i}")
                     for i in range(2)]
            cst = {}
            for nm, shp in [("e2sel", [2, 128]), ("ident", [128, 128]),
                            ("triA", [128, 128]), ("triA2", [128, 128]),
                            ("bc0", [128, 2 * QB]), ("bc1", [128, 2 * QB]),
                            ("bw0", [128, 2 * QB]), ("bw1", [128, 2 * QB]),
                            ("ones64", [1, 64]), ("ones64c", [64, 1])]:
                cst[nm] = pp.tile(shp, BF, tag=nm, name=nm)
            # Phase-A constants now; attention-only constants (masks, wo)
            # are DMA'd at the start of Phase B to keep the head free for x
            for nm in ("e2sel", "ident", "ones64c", "ones64"):
                nc.sync.dma_start(cst[nm][:], d[nm][:])
            eps_sb = pp.tile([128, 1], F32, tag="eps")
            nc.vector.memset(eps_sb[:], EPS)

            # =================================================================
            # Phase A: projections + rope + rmsnorm + vaug build
            # =================================================================
            with (
                tc.tile_pool(name="xp", bufs=1) as xp,
                tc.tile_pool(name="work", bufs=1) as wk,
                tc.tile_pool(name="trig", bufs=1) as trg,
                tc.tile_pool(name="pj_ps", bufs=2, space="PSUM") as pjp,
                tc.tile_pool(name="sw_ps", bufs=2, space="PSUM") as swp,
                tc.tile_pool(name="aux_ps", bufs=1, space="PSUM") as axp,
            ):
                xsb = [xp.tile([128, T], BF, tag=f"x{e}", name=f"x{e}")
                       for e in range(8)]
                wq_sb = [xp.tile([128, 256], BF, tag=f"wq{e}", name=f"wqs{e}")
                         for e in range(8)]
                wkv_sb = [xp.tile([128, 128], BF, tag=f"wkv{e}", name=f"wkvs{e}")
                          for e in range(8)]
                # weights + x first, split into [32, T] row-slices so all 16
                # DMA queues load-balance and x lands as early as possible
                for e in range(8):
                    nc.sync.dma_start(wq_sb[e][:], d["wq"][ds(128 * e, 128), :])
                    nc.sync.dma_start(wkv_sb[e][:], d["wkv"][ds(128 * e, 128), :])
                wg_sb = sm.tile([GC, 1], BF, tag="wg")
                nc.sync.dma_start(wg_sb[:], d["wg"][:])
                aux = {}
                for nm, shp in [("pswq", [128, 128]), ("pswkv", [128, 128]),
                                ("bdq", [128, 2]), ("bdk", [128, 1])]:
                    aux[nm] = xp.tile(shp, BF, tag=nm, name=f"aux_{nm}")
                    nc.sync.dma_start(aux[nm][:], d[nm][:])
                for e in range(8):
                    nc.sync.dma_start(xsb[e][:], d["xT"][ds(128 * e, 128), :])
                ve_sb = xp.tile([128, NKB, HD], BF, tag="ve")
                nc.sync.dma_start(
                    ve_sb[:], d["ve"][:, :].rearrange("(n p) d -> p n d", p=128))

                # gate: u = x[:, :GC] @ wg ; g2 = sigmoid(u) (ve pre-doubled)
                gate_ps = axp.tile([128, NKB], F32, tag="aux")
                for kb in range(NKB):
                    nc.tensor.matmul(
                        gate_ps[:, ds(kb, 1)],
                        xsb[0][0:GC, ts(kb, 128)], wg_sb[:],
                        start=True, stop=True)
                g2 = xp.tile([128, NKB], F32, tag="g2")
                nc.scalar.activation(g2[:], gate_ps[:], AF.Sigmoid)

                # Phase A is software-pipelined across the three projection
                # calls (kv, q-pair0, q-pair1): stage_a is the big PE block
                # (projection + rope swap matmuls); the DVE/scalar-heavy
                # rms + scale tails hide under the next call's stage_a.
                # The k rmsnorm never touches k itself: it is folded into the
                # exp() of Phase B as a per-k-token (per-partition) scale.
                def stage_a(widx, w_tiles, mcols, psw, cos_t, sin_t,
                            raw=None, sq_rows=128):
                    if raw is None:
                        raw = wk.tile([128, T], BF, tag=f"w0{widx}", bufs=1,
                                      name=f"raw{widx}")
                    t1 = wk.tile([128, T], BF, tag=f"w1{widx}", bufs=1,
                                 name=f"t1{widx}")
                    tmp2 = wk.tile([128, T], BF, tag=f"w2{widx}", bufs=1,
                                   name=f"tmp2{widx}")
                    for nchk in range(4):
                        cols = ds(512 * nchk, 512)
                        ps = pjp.tile([128, 512], F32, tag="pj")
                        for e in range(8):
                            nc.tensor.matmul(
                                ps[:], w_tiles[e][:, mcols],
                                xsb[e][:, cols],
                                start=(e == 0), stop=(e == 7))
                        nc.any.tensor_copy(raw[:, cols], ps[:])
                    # rope: roped = raw*cos + (psw @ raw)*sin   (in place)
                    nc.vector.tensor_mul(t1[:], raw[:], cos_t[:])
                    for nchk in range(4):
                        cols = ds(512 * nchk, 512)
                        sw = swp.tile([128, 512], F32, tag="sw")
                        mi = nc.tensor.matmul(sw[:], psw[:], raw[:, cols],
                                              start=True, stop=True)
                        if nchk > 0:
                            _no_ldw(mi)
                        nc.vector.tensor_mul(tmp2[:, cols], sw[:],
                                             sin_t[:, cols])
                    roped = raw
                    nc.vector.tensor_add(roped[:], t1[:], tmp2[:])
                    sq = t1
                    nc.vector.tensor_mul(sq[0:sq_rows, :],
                                         roped[0:sq_rows, :],
                                         roped[0:sq_rows, :])
                    return roped, sq

                def k_normalize(kv_sq):
                    """Normalize kvfin's k rows by rsqrt(mean k^2 + eps).
                    rms stats are computed token-major ([128, NKB] - cheap
                    lane-parallel Ln/Exp), PE-transposed back into a [1, T]
                    row via identity extraction, broadcast to 64 partitions,
                    then multiplied into kvfin in place."""
                    msk = axp.tile([128, NKB], F32, tag="aux")
                    for kb in range(NKB):
                        nc.tensor.matmul(
                            msk[:, ds(kb, 1)],
                            kv_sq[0:64, ts(kb, 128)], cst["ones64c"][:],
                            start=True, stop=True)
                    lnk = sm.tile([128, NKB], F32, tag="lnk", bufs=1)
                    nc.scalar.activation(lnk[:], msk[:], AF.Ln,
                                         scale=1.0 / HD, bias=eps_sb[:])
                    rskt = xp.tile([128, NKB], BF, tag="rskt")
                    nc.scalar.activation(rskt[:], lnk[:], AF.Exp, scale=-0.5)
                    rs_row = xp.tile([1, T], BF, tag="rsrow")
                    for nchk in range(4):
                        cols = ds(512 * nchk, 512)
                        rx = axp.tile([1, 512], F32, tag="aux")
                        for j in range(4):
                            nc.tensor.matmul(
                                rx[0:1, ds(128 * j, 128)],
                                rskt[:, ds(4 * nchk + j, 1)], cst["ident"][:],
                                start=True, stop=True)
                        nc.any.tensor_copy(rs_row[0:1, cols], rx[:])
                    for nchk in range(4):
                        cols = ds(512 * nchk, 512)
                        rsb = swp.tile([128, 512], F32, tag="sw")
                        nc.tensor.matmul(rsb[0:64, :], cst["ones64"][:],
                                         rs_row[0:1, cols],
                                         start=True, stop=True)
                        nc.vector.tensor_mul(kvfin[0:64, cols],
                                             kvfin[0:64, cols], rsb[0:64, :])

                def stage_bc_q(i, roped, sq):
                    """per-512-chunk: rms stats -> rsqrt row -> broadcast ->
                    scaled bf16 heads into qall (chunk-pipelined)."""
                    for nchk in range(4):
                        cols = ds(512 * nchk, 512)
                        msps = axp.tile([2, 512], F32, tag="aux")
                        nc.tensor.matmul(msps[:], aux["bdq"][:, 0:2],
                                         sq[:, cols], start=True, stop=True)
                        lnm = sm.tile([2, 512], F32, tag="lnm", bufs=2)
                        nc.scalar.activation(lnm[:], msps[:], AF.Ln,
                                             scale=1.0 / HD,
                                             bias=eps_sb[0:2, :])
                        rsc = sm.tile([2, 512], BF, tag="rsc", bufs=2)
                        nc.scalar.activation(rsc[:], lnm[:], AF.Exp,
                                             scale=-0.5)
                        rsb = swp.tile([128, 512], F32, tag="sw")
                        nc.tensor.matmul(rsb[:], cst["e2sel"][:], rsc[:],
                                         start=True, stop=True)
                        for hl in range(2):
                            nc.vector.tensor_mul(
                                qall[:, 2 * i + hl, cols],
                                roped[ds(64 * hl, 64), cols],
                                rsb[ds(64 * hl, 64), :])

                def build_vaug():
                    for kb in range(NKB):
                        vt = pjp.tile([128, HD], BF, tag="pj", bufs=2)
                        nc.tensor.transpose(vt[:], kvfin[64:128, ts(kb, 128)],
                                            cst["ident"][64:128, 64:128])
                        gv = sm.tile([128, HD], BF, tag="gv")
                        nc.vector.tensor_scalar_mul(gv[:], ve_sb[:, kb, :],
                                                    g2[:, ds(kb, 1)])
                        nc.vector.memset(vaug[kb][:, ds(HD, 1)], 1.0)
                        nc.vector.tensor_add(vaug[kb][:, 0:HD], gv[:], vt[:])

                cos_kv = trg.tile([128, T], BF, tag="tckv")
                sin_kv = trg.tile([128, T], BF, tag="tskv")
                cos_q = trg.tile([128, T], BF, tag="tcq")
                sin_q = trg.tile([128, T], BF, tag="tsq")
                for tile_, nm in [(cos_kv, "coskv"), (sin_kv, "sinkv"),
                                  (cos_q, "cos4"), (sin_q, "sin4")]:
                    nc.sync.dma_start(tile_[:], d[nm][:])

                kv_roped, kv_sq = stage_a(2, wkv_sb, ds(0, 128),
                                          aux["pswkv"], cos_kv, sin_kv,
                                          raw=kvfin, sq_rows=64)
                q0_roped, q0_sq = stage_a(0, wq_sb, ds(0, 128),
                                          aux["pswq"], cos_q, sin_q)
                k_normalize(kv_sq)
                build_vaug()
                q1_roped, q1_sq = stage_a(1, wq_sb, ds(128, 128),
                                          aux["pswq"], cos_q, sin_q)
                stage_bc_q(0, q0_roped, q0_sq)
                stage_bc_q(1, q1_roped, q1_sq)

            # =================================================================
            # Phase B: attention + output projection
            # =================================================================
            with (
                tc.tile_pool(name="big_ps", bufs=4, space="PSUM") as bigp,
                tc.tile_pool(name="yt_ps", bufs=2, space="PSUM") as ytp,
                tc.tile_pool(name="et", bufs=4) as etp,
                tc.tile_pool(name="stage", bufs=2) as stg,
            ):
                for nm in ("triA", "triA2", "bc0", "bc1", "bw0", "bw1"):
                    nc.sync.dma_start(cst[nm][:], d[nm][:])
                for i in range(2):
                    nc.sync.dma_start(wo_sb[i][:], d["wo"][ds(128 * i, 128), :])
                def mask_for(qb, kb):
                    if kb == 2 * qb:
                        return (cst["triA"], cst["bc0"])
                    if kb == 2 * qb + 1:
                        return (cst["triA"], cst["bc1"])
                    if kb == 2 * qb - 8:
                        return (cst["triA2"], cst["bw0"])
                    if kb == 2 * qb - 7:
                        return (cst["triA2"], cst["bw1"])
                    return None

                def make_tail(qb, yts):
                    """Denominator + ytall writes for qb (emitted inside the
                    next qb's score stream so the PE never idles on it)."""
                    qsl = ds(QB * qb, QB)

                    def tail():
                        # Z row -> bf16 SBUF, PE-broadcast to 64 partitions,
                        # 1/Z = exp(-ln(Z)) on the lane-parallel scalar engine
                        zrow = sm.tile([1, 4, QB], BF, tag="zrow", bufs=2)
                        nc.vector.tensor_copy(zrow[:], yts[ds(HD, 1), :, :])
                        for p in range(2):
                            rbt = bigp.tile([128, 2, QB], F32, tag="big",
                                            name=f"rb{qb}_{p}")
                            nc.tensor.matmul(rbt[0:64, :, :],
                                             cst["ones64"][:],
                                             zrow[:, ds(2 * p, 2), :],
                                             start=True, stop=True)
                            lnz = stg.tile([64, 2, QB], F32, tag="lnz",
                                           bufs=2)
                            nc.scalar.activation(lnz[:], rbt[0:64, :, :],
                                                 AF.Ln)
                            rinv = stg.tile([64, 2, QB], F32, tag="rinv",
                                            bufs=2)
                            nc.scalar.activation(rinv[:], lnz[:], AF.Exp,
                                                 scale=-1.0)
                            for hl in range(2):
                                h = 2 * p + hl
                                nc.vector.tensor_mul(
                                    ytall[h // 2][ds(64 * (h % 2), 64), qsl],
                                    yts[0:HD, h, :], rinv[:, hl, :])

                    def outp(tt):
                        for nn in range(2):
                            po = bigp.tile([128, 2, QB], F32, tag="big",
                                           name=f"po{tt}_{nn}")
                            for i in range(2):
                                nc.tensor.matmul(
                                    po[:],
                                    ytall[i][:, ts(tt, 128)],
                                    wo_sb[i][:, ds(512 * nn, 512)],
                                    start=(i == 0), stop=(i == 1))
                            osb = stg.tile([128, 2, QB], F32, tag="osb",
                                           bufs=4)
                            nc.vector.tensor_copy(osb[:], po[:])
                            nc.sync.dma_start(
                                out_d[ts(tt, 128), ds(512 * nn, 512)].rearrange(
                                    "p (n c) -> p n c", n=2), osb[:])

                    return [tail, lambda: outp(2 * qb),
                            lambda: outp(2 * qb + 1)]

                pending = []
                for qb in range(NQB):
                    kbs = list(range(max(0, 2 * qb - 8), 2 * qb + 2))
                    qsl = ds(QB * qb, QB)
                    yts = ytp.tile([HD + 1, 4, QB], F32, tag="yts",
                                   name=f"yts{qb}")

                    def emit_scores(kb):
                        mask = mask_for(qb, kb)
                        et = etp.tile([128, 4, QB], BF, tag="et")
                        for p in range(2):
                            sc = bigp.tile([128, 2, QB], F32, tag="big",
                                           name=f"sc{qb}_{kb}_{p}")
                            nc.tensor.matmul(
                                sc[:],
                                kvfin[0:64, ts(kb, 128)],
                                qall[:, ds(2 * p, 2), qsl],
                                start=True, stop=(mask is None))
                            if mask is not None:
                                nc.tensor.matmul(
                                    sc[:], mask[0][:], mask[1][:],
                                    start=False, stop=True)
                            nc.scalar.activation(et[:, ds(2 * p, 2), :],
                                                 sc[:], AF.Exp, scale=SCALE)
                        return et

                    def emit_pv(kb, et):
                        for p in range(2):
                            mi = nc.tensor.matmul(
                                yts[:, ds(2 * p, 2), :], vaug[kb][:],
                                et[:, ds(2 * p, 2), :],
                                start=(kb == kbs[0]), stop=(kb == kbs[-1]))
                            if p == 1:
                                _no_ldw(mi)

                    # depth-1 PV pipeline within qb; the previous qb's tail
                    # and output projections flush between early units here
                    window = []
                    for idx, kb in enumerate(kbs):
                        et = emit_scores(kb)
                        if 1 <= idx <= 3 and pending:
                            pending.pop(0)()
                        window.append((kb, et))
                        if len(window) > 1:
                            emit_pv(*window.pop(0))
                    while pending:
                        pending.pop(0)()
                    for unit in window:
                        emit_pv(*unit)
                    pending = make_tail(qb, yts)
                for fn in pending:
                    fn()

    return nc


# ---------------------------------------------------------------------------
# walrus workaround: this build rejects >1 sync-wait on CTRL-class ops
# (e.g. the Tile tail Drain). Move excess waits onto NOPs inserted before.
# ---------------------------------------------------------------------------
_CTRL_TYPES = (mybir.InstDrain, mybir.InstNoOp, mybir.InstEventSemaphore)


def _split_excess_waits(nc, limit=1):
    for fn in nc.m.functions:
        for bb in fn.blocks:
            out, changed = [], False
            for inst in bb.instructions:
                si = inst.sync_info
                waits = list(si.on_wait) if si is not None and si.on_wait else []
                if len(waits) > limit:
                    extra, keep = waits[:-limit], waits[-limit:]
                    while extra:
                        chunk, extra = extra[:limit], extra[limit:]
                        nop = mybir.InstNoOp(
                            name=f"{inst.name}-wsplit{len(out)}", ins=[],
                            outs=[])
                        nop.engine = inst.engine
                        nop.sync_info = mybir.SyncInfo(on_wait=chunk,
                                                       on_update=[])
                        out.append(nop)
                    si.on_wait = keep
                    inst.sync_info = si
                    changed = True
                out.append(inst)
            if changed:
                bb.instructions = out


# ---------------------------------------------------------------------------
# Host-side constants (shared by all cores)
# ---------------------------------------------------------------------------
_BF_NP = mybir.dt.np(BF)


def _bf(a):
    return np.asarray(a, dtype=_BF_NP)


def _host_constants():
    c = {}
    m = np.arange(128)[:, None]
    j = np.arange(128)[None, :]
    i = np.arange(QB)[None, :]
    c["triA"] = _bf(m <= j)                          # causal counting lhsT
    c["triA2"] = _bf(m >= j)                         # window counting lhsT
    bc0 = np.where(m > i, -BIG, 0.0)
    bc1 = np.where(m > i - 128, -BIG, 0.0)
    bw0 = np.where(m < i, -BIG, 0.0)
    bw1 = np.where(m + 128 < i, -BIG, 0.0)
    for nm, v in [("bc0", bc0), ("bc1", bc1), ("bw0", bw0), ("bw1", bw1)]:
        c[nm] = _bf(np.tile(v, (1, 2)))              # 2 heads per matmul
    sw = np.zeros((128, 128), np.float32)            # pswq[f, m]=1 iff f=sig(m)
    for mm in range(128):
        f = mm + 32 if (mm % 64) < 32 else mm - 32
        sw[f, mm] = 1.0
    c["pswq"] = _bf(sw)
    swkv = sw.copy()
    swkv[:, 64:] = 0.0
    c["pswkv"] = _bf(swkv)
    bdq = np.zeros((128, 2), np.float32)
    bdq[0:64, 0] = 1.0
    bdq[64:128, 1] = 1.0
    c["bdq"] = _bf(bdq)
    bdk = np.zeros((128, 1), np.float32)
    bdk[0:64, 0] = 1.0
    c["bdk"] = _bf(bdk)
    e2 = np.zeros((2, 128), np.float32)
    e2[0, 0:64] = 1.0
    e2[1, 64:128] = 1.0
    c["e2sel"] = _bf(e2)
    c["ident"] = _bf(np.eye(128))
    c["ones64"] = _bf(np.ones((1, 64)))
    c["ones64c"] = _bf(np.ones((64, 1)))
    return c


def _trig(cos_b, sin_b):
    """cos_b/sin_b: [T, HD//2] -> the four [128, T] rope coefficient maps."""
    ct = np.ascontiguousarray(cos_b.T)               # [32, T]
    st = np.ascontiguousarray(sin_b.T)
    cos4 = np.tile(ct, (4, 1))                       # [c;c;c;c]
    sin4 = np.tile(np.concatenate([st, -st], 0), (2, 1))
    coskv = np.concatenate([ct, ct, np.ones((64, T), np.float32)], 0)
    sinkv = np.concatenate([st, -st, np.zeros((64, T), np.float32)], 0)
    return _bf(cos4), _bf(sin4), _bf(coskv), _bf(sinkv)


# ---------------------------------------------------------------------------
# Cached PJRT runner (compile once per process)
# ---------------------------------------------------------------------------
_RUNNER = None


def _get_runner():
    global _RUNNER
    if _RUNNER is not None:
        return _RUNNER
    import jax
    from jax.experimental.shard_map import shard_map
    from jax.sharding import Mesh, PartitionSpec
    from concourse.bass2jax import (_bass_exec_p, install_neuronx_cc_hook,
                                    partition_id_tensor)

    nc = _build_nc()
    _split_excess_waits(nc)
    install_neuronx_cc_hook()

    pid_name = (nc.partition_id_tensor.name
                if nc.partition_id_tensor is not None else None)
    in_names, out_names, out_avals, zero_outs = [], [], [], []
    for alloc in nc.m.functions[0].allocations:
        if not isinstance(alloc, mybir.MemoryLocationSet):
            continue
        name = alloc.memorylocations[0].name
        if alloc.kind == "ExternalInput":
            if name == pid_name:
                continue
            in_names.append(name)
        elif alloc.kind == "ExternalOutput":
            np_dt = mybir.dt.np(alloc.dtype)
            out_names.append(name)
            out_avals.append(
                jax.core.ShapedArray(tuple(alloc.tensor_shape), np_dt))
            zero_outs.append(
                np.zeros(tuple(alloc.tensor_shape), np_dt))

    def _body(*args):
        operands = list(args)
        if pid_name is not None:
            operands.append(partition_id_tensor())
        outs = _bass_exec_p.bind(
            *operands,
            out_avals=tuple(out_avals),
            in_names=(tuple(in_names) + tuple(out_names)
                      + ((pid_name,) if pid_name else ())),
            out_names=tuple(out_names),
            lowering_input_output_aliases=(),
            sim_require_finite=True,
            sim_require_nnan=True,
            nc=nc,
        )
        return tuple(outs)

    devices = jax.devices()[:NCORES]
    mesh = Mesh(np.asarray(devices), ("core",))
    n_args = len(in_names) + len(out_names)
    sharded = jax.jit(
        shard_map(_body, mesh=mesh,
                  in_specs=(PartitionSpec("core"),) * n_args,
                  out_specs=(PartitionSpec("core"),) * len(out_names),
                  check_rep=False),
        keep_unused=True,
    )

    def run(in_maps):
        concat_in = [
            np.concatenate([in_maps[c][nm] for c in range(NCORES)], axis=0)
            for nm in in_names
        ]
        concat_zero = [
            np.zeros((NCORES * z.shape[0], *z.shape[1:]), z.dtype)
            for z in zero_outs
        ]
        outs = sharded(*concat_in, *concat_zero)
        res = []
        for c in range(NCORES):
            res.append({
                nm: np.asarray(outs[i]).reshape(NCORES, *out_avals[i].shape)[c]
                for i, nm in enumerate(out_names)
            })
        return res

    _RUNNER = {"run": run, "sharded": sharded, "in_names": in_names,
               "out_names": out_names, "out_avals": out_avals,
               "zero_outs": zero_outs, "nc": nc, "mesh": mesh}
    return _RUNNER


def _make_in_maps(x, ve, cos, sin, Wq, Wk, Wv, Wo, Wg):
    cstc = _host_constants()
    in_maps = []
    for c in range(NCORES):
        b, g = c // 4, c % 4
        cos4, sin4, coskv, sinkv = _trig(np.asarray(cos[b]),
                                         np.asarray(sin[b]))
        m = {
            "xT": _bf(np.asarray(x[b]).T),
            # gate = 2*sigmoid(..): the 2x is folded into ve here
            "ve": _bf(2.0 * np.asarray(ve[b])[:, HD * g:HD * (g + 1)]),
            "cos4": cos4, "sin4": sin4, "coskv": coskv, "sinkv": sinkv,
            "wq": _bf(Wq[:, 256 * g:256 * (g + 1)]),
            "wkv": _bf(np.concatenate([Wk[:, HD * g:HD * (g + 1)],
                                       Wv[:, HD * g:HD * (g + 1)]], axis=1)),
            "wg": _bf(Wg[:, g:g + 1]),
            "wo": _bf(Wo[256 * g:256 * (g + 1), :]),
        }
        m.update(cstc)
        in_maps.append(m)
    return in_maps


def kernel(x, ve, cos, sin, Wq, Wk, Wv, Wo, Wg, window_size):
    assert int(window_size) == WIN, f"kernel hardcodes window={WIN}"
    x, ve, cos, sin = (np.asarray(a, np.float32) for a in (x, ve, cos, sin))
    Wq, Wk, Wv, Wo, Wg = (np.asarray(a, np.float32)
                          for a in (Wq, Wk, Wv, Wo, Wg))
    runner = _get_runner()
    in_maps = _make_in_maps(x, ve, cos, sin, Wq, Wk, Wv, Wo, Wg)
    res = runner["run"](in_maps)
    out = np.zeros((B, T, NE), np.float32)
    for c in range(NCORES):
        out[c // 4] += res[c]["out"]
    return out
